# revision 5
# baseline (speedup 1.0000x reference)
"""GNN linear-attention kernel for Trainium2 over an axon-tunneled PJRT client.

The device compute for this problem (~2 GFLOP/graph) is trivial next to the
cost of moving data through the tunnel (~30-60 MB/s, ~90 ms/RPC), so the
kernel is organized entirely around the data path:

  - A is a 0/1 adjacency matrix: pack to 1 bit/element on the host
    (134 MB f32 -> 4.2 MB) and unpack on-device.
  - x and the weights ship as one f16 payload per chunk; the output returns
    as f16 and is upcast on the host (rel-err ~3e-4, gate is 2e-2).
  - The batch is split into chunks; uploads, device execution, and downloads
    of different chunks overlap via threads (the tunnel is full-duplex).
  - Everything runs on one NeuronCore: per-op RPC latency dominates any
    benefit of spreading trivial compute across 8 cores.
"""

import numpy as np
from concurrent.futures import ThreadPoolExecutor

B, N, D, O = 8, 2048, 128, 128
NCHUNKS = 4
CHUNK = B // NCHUNKS
NW = 3 * D * D + 4 * D  # f16 elements of packed weights per chunk payload

_state = {}


def _get_state():
    if _state:
        return _state
    import jax
    import jax.numpy as jnp

    dev = jax.devices()[0]

    def chunk_compute(bits, xw):
        # bits: (g, N, N//8) u8;  xw: flat f16 = [weights | x chunk]
        i = 0

        def take(n, shape):
            nonlocal i
            t = xw[i : i + n].astype(jnp.float32).reshape(shape)
            i += n
            return t

        W_qk = take(D * D, (D, D))
        W_l = take(D * O, (D, O))
        W_r = take(D * O, (D, O))
        b_qk = take(D, (D,))
        b_l = take(O, (O,))
        W_d = take(D, (1, D))
        b_d = take(D, (D,))
        x = xw[i:].astype(jnp.float32).reshape(CHUNK, N, D)

        shifts = jnp.arange(7, -1, -1, dtype=jnp.uint8)
        A = (bits[..., None] >> shifts) & jnp.uint8(1)
        A = A.reshape(CHUNK, N, N).astype(jnp.float32)
        deg = jnp.sum(A, axis=-1, keepdims=True)
        gate = jax.nn.sigmoid(deg @ W_d + b_d)
        xg = x * gate
        QK = jax.nn.sigmoid(xg @ W_qk + b_qk)
        scores = jnp.einsum("bnd,bmd->bnm", QK, QK) / jnp.sqrt(jnp.float32(D))
        scores = scores * A
        attn = scores / (jnp.sum(scores, axis=-1, keepdims=True) + 1e-6)
        agg = jnp.einsum("bnm,bmd->bnd", attn, xg)
        out = agg @ W_l + b_l + xg @ W_r
        nrm = jnp.linalg.norm(out, axis=-1, keepdims=True)
        out = out / jnp.maximum(nrm, 1e-12)
        # 12-bit fixed-point pack: rows are unit-normalized so |v| <= 1.
        # Two 12-bit values -> 3 bytes; 25% fewer bytes on the (slow) down path.
        q = jnp.clip(jnp.round(out * 2047.0), -2047, 2047).astype(jnp.int32) + 2048
        q = q.astype(jnp.uint16).reshape(CHUNK, N, O // 2, 2)
        q0 = q[..., 0].astype(jnp.uint32)
        q1 = q[..., 1].astype(jnp.uint32)
        b0 = q0 & 0xFF
        b1 = (q0 >> 8) | ((q1 & 0xF) << 4)
        b2 = q1 >> 4
        return jnp.stack([b0, b1, b2], axis=-1).astype(jnp.uint8)

    _state["jax"] = jax
    _state["dev"] = dev
    _state["fn"] = jax.jit(chunk_compute)
    _state["pool"] = ThreadPoolExecutor(max_workers=8)
    return _state


def _fast_path(x, A, W_qk, b_qk, W_l, b_l, W_r, W_d, b_d):
    st = _get_state()
    jax, dev, fn, pool = st["jax"], st["dev"], st["fn"], st["pool"]

    w16 = np.concatenate(
        [
            np.ascontiguousarray(W_qk, np.float32).reshape(-1),
            np.ascontiguousarray(W_l, np.float32).reshape(-1),
            np.ascontiguousarray(W_r, np.float32).reshape(-1),
            np.ascontiguousarray(b_qk, np.float32).reshape(-1),
            np.ascontiguousarray(b_l, np.float32).reshape(-1),
            np.ascontiguousarray(W_d, np.float32).reshape(-1),
            np.ascontiguousarray(b_d, np.float32).reshape(-1),
        ]
    ).astype(np.float16)
    assert w16.size == NW

    def put(arr):
        return jax.device_put(arr, dev)

    # A viewed as bytes: a 0/1 f32 element is nonzero exactly in its top byte,
    # and np.packbits packs any-nonzero as 1, so pack the strided byte view
    # directly (no bool temp).
    Ab = A.view(np.uint8).reshape(B, N, N, 4)

    ys = []
    fetches = []

    def fetch(y):
        b = np.asarray(y).astype(np.uint16)  # (g, N, O//2, 3)
        q0 = (b[..., 0] | ((b[..., 1] & 0xF) << 8)).astype(np.int32)
        q1 = ((b[..., 1] >> 4) | (b[..., 2] << 4)).astype(np.int32)
        q = np.stack([q0, q1], axis=-1).reshape(CHUNK, N, O)
        return (q - 2048).astype(np.float32) * (1.0 / 2047.0)

    for c in range(B // CHUNK):
        sl = slice(c * CHUNK, (c + 1) * CHUNK)
        xw = np.concatenate([w16, x[sl].astype(np.float16).reshape(-1)])
        xw_fut = pool.submit(put, xw)
        bits = np.packbits(Ab[sl, :, :, 3], axis=-1)
        bits_fut = pool.submit(put, bits)
        y = fn(bits_fut.result(), xw_fut.result())
        try:
            y.copy_to_host_async()
        except Exception:
            pass
        ys.append(y)
        fetches.append(pool.submit(fetch, y))

    result = np.empty((B, N, O), np.float32)
    for c, f in enumerate(fetches):
        result[c * CHUNK : (c + 1) * CHUNK] = f.result()
    return result


def _fallback(x, A, W_qk, b_qk, W_l, b_l, W_r, W_d, b_d):
    import jax
    import jax.numpy as jnp

    if "fb" not in _state:

        def f(x_b, A_b, W_qk, b_qk, W_l, b_l, W_r, W_d, b_d):
            deg = jnp.sum(A_b, axis=-1, keepdims=True)
            gate = jax.nn.sigmoid(deg @ W_d + b_d)
            xg = x_b * gate
            QK = jax.nn.sigmoid(xg @ W_qk + b_qk)
            scores = (QK @ QK.T) / jnp.sqrt(jnp.float32(D))
            scores = scores * A_b
            attn = scores / (jnp.sum(scores, axis=-1, keepdims=True) + 1e-6)
            agg = attn @ xg
            out = agg @ W_l + b_l + xg @ W_r
            nrm = jnp.linalg.norm(out, axis=-1, keepdims=True)
            return out / jnp.maximum(nrm, 1e-12)

        _state["fb"] = jax.jit(f)
    fn = _state["fb"]
    dev = jax.devices()[0]
    ws = [jax.device_put(np.asarray(t), dev) for t in (W_qk, b_qk, W_l, b_l, W_r, W_d, b_d)]
    out = np.stack(
        [np.asarray(fn(jax.device_put(x[b], dev), jax.device_put(A[b], dev), *ws)) for b in range(B)]
    )
    return out.astype(np.float32)


def kernel(x, A, W_qk, b_qk, W_l, b_l, W_r, W_d, b_d):
    x = np.ascontiguousarray(x, np.float32)
    A = np.ascontiguousarray(A, np.float32)
    try:
        return _fast_path(x, A, W_qk, b_qk, W_l, b_l, W_r, W_d, b_d)
    except Exception:
        return _fallback(x, A, W_qk, b_qk, W_l, b_l, W_r, W_d, b_d)
